# revision 1
# baseline (speedup 1.0000x reference)
"""Masked dot-product attention on 8 Trainium2 NeuronCores (Bass/Tile).

Problem: B=16, LQ=LK=2048, D=128 fp32; per-batch key valid_lens mask.
Sharding: 64 (batch, 512-query-block) units bin-packed into 8 slots x 8
cores by per-batch valid k-tile count, so every core runs an identical
(SPMD) program while skipping masked-out key tiles entirely.

Per unit, on device (scores kept transposed so no probability transpose
is ever needed):
  S^T[k, q]  = KT_tile.T @ QT_block       (fp32r matmuls, N=512, full rate)
  W^T        = exp(S^T/sqrt(D) + bias[k]) (ScalarE; bias -1e4 masks invalid
                                           key rows -> exp underflows to 0)
  rowsum[q] += ones.T @ W^T               (PE, M=1, bf16, PSUM accumulate)
  O^T[d, q] += V_tile.T @ W^T             (PE, bf16, PSUM accumulate)
then O^T * (1/rowsum) (GPSIMD partition-broadcast + DVE), PE-transpose
back to [q, d], evict via ScalarE, DMA out. bf16 is used for the
probability/value matmuls because fp32r accumulating matmuls
(start=False) measured ~15x slower on hardware.
"""

import math

import ml_dtypes
import numpy as np

import concourse.bass as bass
import concourse.mybir as mybir
import concourse.tile as tile
from concourse import bacc
from concourse.bass_utils import run_bass_kernel_spmd
from concourse.masks import make_identity

B, LQ, LK, D = 16, 2048, 2048, 128
N_CORES = 8
QB = 512          # query block (one unit) = QB rows of Q
N_SLOTS = (LQ // QB) * B // N_CORES   # 8 slots per core
KT = 128          # key tile
F32 = mybir.dt.float32
F32R = mybir.dt.float32r
BF16 = mybir.dt.bfloat16
FP16 = mybir.dt.float16
SCALE = 1.0 / math.sqrt(D)


def _plan(valid_lens):
    """Assign 64 (batch, qblock) units to an 8x8 (slot, core) grid.

    Returns (slot_units, slot_ntiles, masked_from) where
      slot_units[s][c] = (batch, qblock) handled by core c in slot s
      slot_ntiles[s]   = k-tiles processed in slot s (max over cores)
      masked_from[s]   = first k-tile index needing a mask multiply
    """
    vl = np.asarray(valid_lens).astype(np.int64)
    ktiles = np.maximum(1, np.ceil(vl / KT).astype(np.int64))
    units = [(int(b), j) for b in range(B) for j in range(LQ // QB)]
    units.sort(key=lambda u: -ktiles[u[0]])
    slot_units, slot_ntiles, masked_from = [], [], []
    for s in range(N_SLOTS):
        chunk = units[s * N_CORES:(s + 1) * N_CORES]
        slot_units.append(chunk)
        slot_ntiles.append(int(max(ktiles[b] for b, _ in chunk)))
        # tile t is fully valid for all cores iff t < min(floor(vl/KT));
        # from there on a multiplicative mask is required on some core.
        masked_from.append(int(min(vl[b] // KT for b, _ in chunk)))
    return slot_units, slot_ntiles, masked_from


def _pack_inputs(queries, keys, values, valid_lens, slot_units, slot_ntiles):
    """Build per-core packed input arrays (host-side numpy)."""
    vl = np.asarray(valid_lens).astype(np.int64)
    qt = np.ascontiguousarray(np.transpose(queries, (0, 2, 1)))   # [B, D, LQ]
    kt = np.ascontiguousarray(np.transpose(keys, (0, 2, 1)))      # [B, D, LK]
    u_total = sum(slot_ntiles)
    in_maps = []
    for c in range(N_CORES):
        qt_p = np.zeros((N_SLOTS, D, QB), np.float16)
        kt_p = np.zeros((u_total, D, KT), np.float16)
        v_p = np.zeros((u_total, KT, D), np.float16)
        mask_p = np.zeros((KT, u_total), np.float32)
        off = 0
        for s in range(N_SLOTS):
            b, j = slot_units[s][c]
            qt_p[s] = qt[b, :, j * QB:(j + 1) * QB]
            n_valid = int(min(slot_ntiles[s], math.ceil(vl[b] / KT)))
            for t in range(n_valid):
                k0 = t * KT
                kt_p[off + t] = kt[b, :, k0:k0 + KT]
                v_p[off + t] = values[b, k0:k0 + KT, :]
                nv = int(min(KT, vl[b] - k0))
                mask_p[:nv, off + t] = 1.0
            off += slot_ntiles[s]
        bias_p = (mask_p - 1.0) * 10000.0
        in_maps.append({
            "qt": qt_p, "kt": kt_p, "v": v_p, "mask": bias_p,
            "ones": np.ones((KT, 1), np.float16),
        })
    return in_maps, u_total


def build_kernel(slot_ntiles, masked_from, u_total, reps=1,
                 do_exp=True, do_rs=True, do_pv=True, do_epi=True):
    nc = bacc.Bacc(None, target_bir_lowering=False, debug=True)
    qt_d = nc.dram_tensor("qt", [N_SLOTS, D, QB], FP16, kind="ExternalInput")
    kt_d = nc.dram_tensor("kt", [u_total, D, KT], FP16, kind="ExternalInput")
    v_d = nc.dram_tensor("v", [u_total, KT, D], FP16, kind="ExternalInput")
    mask_d = nc.dram_tensor("mask", [KT, u_total], F32, kind="ExternalInput")
    ones_d = nc.dram_tensor("ones", [KT, 1], FP16, kind="ExternalInput")
    out_d = nc.dram_tensor("out", [N_SLOTS, QB, D], F32, kind="ExternalOutput")

    G = 2  # k-tiles per exp group (PSUM banks: st 2x2 + ot 2 + rs 1 + to 1 = 8)

    with tile.TileContext(nc) as tc:
        with (
            tc.tile_pool(name="const", bufs=1) as const,
            tc.tile_pool(name="wt_pool", bufs=5) as wt_pool,
            tc.tile_pool(name="ws_pool", bufs=5) as ws_pool,
            tc.tile_pool(name="onorm_pool", bufs=2) as onorm_pool,
            tc.tile_pool(name="ostage_pool", bufs=2) as ostage_pool,
            tc.tile_pool(name="recip_pool", bufs=2) as recip_pool,
            tc.tile_pool(name="bc_pool", bufs=2) as bc_pool,
            tc.tile_pool(name="st_psum", bufs=2, space="PSUM") as st_psum,
            tc.tile_pool(name="ot_psum", bufs=2, space="PSUM") as ot_psum,
            tc.tile_pool(name="rs_psum", bufs=1, space="PSUM") as rs_psum,
            tc.tile_pool(name="to_psum", bufs=1, space="PSUM") as to_psum,
        ):
            identity = const.tile([128, 128], FP16)
            make_identity(nc, identity)
            ones = const.tile([128, 1], FP16)
            nc.sync.dma_start(out=ones, in_=ones_d[:, :])
            qt_all = const.tile([128, N_SLOTS, QB], FP16)
            for s0 in range(0, N_SLOTS, 2):
                nc.sync.dma_start(
                    out=qt_all[:, s0:s0 + 2, :],
                    in_=qt_d[s0:s0 + 2].rearrange("s d q -> d s q"),
                )
            mask_all = const.tile([128, u_total], F32)
            nc.sync.dma_start(out=mask_all, in_=mask_d[:, :])
            kt_all = const.tile([128, u_total, KT], FP16)
            v_all = const.tile([128, u_total, D], FP16)
            bnds = [0, 2, 6] + [
                round(6 + i * (u_total - 6) / 6) for i in range(1, 7)]
            for lo, hi in zip(bnds[:-1], bnds[1:]):
                nc.sync.dma_start(
                    out=kt_all[:, lo:hi, :],
                    in_=kt_d[lo:hi].rearrange("u d k -> d u k"))
                nc.sync.dma_start(
                    out=v_all[:, lo:hi, :],
                    in_=v_d[lo:hi].rearrange("u k d -> k u d"))

            # reps>1 repeats computation for timing isolation (same output)
            pending_epi = []
            for _rep in range(reps):
                off = 0
                for s in range(len(slot_ntiles)):
                    u_s = slot_ntiles[s]
                    ot = ot_psum.tile([128, QB], F32)          # O^T accum [d, q]
                    rs = rs_psum.tile([1, QB], F32)            # rowsum [1, q]
                    qt_s = qt_all[:, s, :]
                    # merged exps below masked_from; per-tile exps with an
                    # additive mask bias (-1e4 -> exp underflows to 0) after
                    groups, t0 = [], 0
                    while t0 < u_s:
                        if t0 < masked_from[s]:
                            gsz = min(G, masked_from[s] - t0, u_s - t0)
                        else:
                            gsz = 1
                        groups.append((t0, gsz))
                        t0 += gsz
                    rs_pend = None
                    rs_emitted = 0
                    for gi, (g, gsz) in enumerate(groups):
                        st = st_psum.tile([128, G, QB], F32)   # S^T group
                        wt = wt_pool.tile([128, G, QB], FP16)   # exp(S^T) group
                        for tt in range(gsz):
                            t = g + tt
                            nc.tensor.matmul(
                                st[:, tt, :], kt_all[:, off + t, :], qt_s,
                                start=True, stop=True,
                            )
                        bias = (0.0 if g < masked_from[s]
                                else mask_all[:, off + g:off + g + 1])
                        nc.scalar.activation(
                            wt[:, :gsz, :], st[:, :gsz, :],
                            mybir.ActivationFunctionType.Exp, scale=SCALE,
                            bias=bias,
                        )
                        if g == 0 and pending_epi:
                            # previous slot's epilogue goes here, behind this
                            # slot's first QK group, so the in-order PE queue
                            # never stalls on the normalize chain
                            pending_epi.pop(0)()
                        if do_rs:
                            # rowsum is linear in the k-tiles: DVE sum-tree
                            # (pair tiles, then pair groups) so one PE rowsum
                            # matmul covers up to 4 k-tiles
                            if gsz == 2:
                                ws = ws_pool.tile([128, QB], FP16)
                                nc.vector.tensor_add(
                                    ws, wt[:, 0, :], wt[:, 1, :])
                                rs_src = ws
                            else:
                                rs_src = wt[:, 0, :]
                            if rs_pend is None and gi < len(groups) - 1:
                                rs_pend = rs_src
                            else:
                                if rs_pend is not None:
                                    ws2 = ws_pool.tile([128, QB], FP16)
                                    nc.vector.tensor_add(
                                        ws2, rs_pend, rs_src)
                                    rs_src = ws2
                                    rs_pend = None
                                nc.tensor.matmul(
                                    rs, ones, rs_src,
                                    start=(rs_emitted == 0),
                                    stop=(gi == len(groups) - 1),
                                )
                                rs_emitted += 1
                        for tt in range(gsz):
                            t = g + tt
                            wt_t = wt[:, tt, :]
                            if do_pv == "noaccum":
                                nc.tensor.matmul(
                                    ot, v_all[:, off + t, :], wt_t,
                                    start=True, stop=True,
                                )
                            elif do_pv:
                                nc.tensor.matmul(
                                    ot, v_all[:, off + t, :], wt_t,
                                    start=(t == 0), stop=(t == u_s - 1),
                                )
                    off += u_s
                    if not do_epi:
                        continue

                    def _epilogue(s=s, ot=ot, rs=rs):
                        recip = recip_pool.tile([1, QB], F32)
                        nc.vector.reciprocal(recip, rs)
                        recip_bc = bc_pool.tile([128, QB], F32)
                        nc.gpsimd.partition_broadcast(recip_bc, recip)
                        onorm = onorm_pool.tile([128, QB], FP16)
                        nc.vector.tensor_mul(onorm, ot, recip_bc)
                        to = to_psum.tile([128, QB], FP16)
                        for j in range(4):
                            nc.tensor.transpose(
                                to[:, j * 128:(j + 1) * 128],
                                onorm[:, j * 128:(j + 1) * 128],
                                identity,
                            )
                        ostage = ostage_pool.tile([128, 4, 128], F32)
                        nc.vector.tensor_copy(ostage, to)
                        nc.sync.dma_start(
                            out=out_d[s].rearrange(
                                "(sub p) d -> p sub d", p=128),
                            in_=ostage,
                        )
                    pending_epi.append(_epilogue)
            for fn in pending_epi:
                fn()
    nc.finalize()
    return nc


def kernel(queries, keys, values, valid_lens):
    queries = np.ascontiguousarray(np.asarray(queries, dtype=np.float32))
    keys = np.ascontiguousarray(np.asarray(keys, dtype=np.float32))
    values = np.ascontiguousarray(np.asarray(values, dtype=np.float32))
    assert queries.shape == (B, LQ, D), queries.shape
    assert keys.shape == (B, LK, D), keys.shape
    assert values.shape == (B, LK, D), values.shape

    slot_units, slot_ntiles, masked_from = _plan(valid_lens)
    in_maps, u_total = _pack_inputs(
        queries, keys, values, valid_lens, slot_units, slot_ntiles)
    nc = build_kernel(slot_ntiles, masked_from, u_total)
    res = None
    last_exc = None
    for attempt in range(3):
        try:
            res = run_bass_kernel_spmd(nc, in_maps, list(range(N_CORES)))
            break
        except Exception as exc:  # transient NRT/axon failures
            last_exc = exc
            try:
                import jax
                jax.clear_caches()
            except Exception:
                pass
    if res is None:
        raise last_exc

    out = np.empty((B, LQ, D), np.float32)
    for c in range(N_CORES):
        o = res.results[c]["out"]
        for s in range(N_SLOTS):
            b, j = slot_units[s][c]
            out[b, j * QB:(j + 1) * QB, :] = o[s]
    return out



# revision 14
# speedup vs baseline: 2.3405x; 2.3405x over previous
"""Masked dot-product attention on 8 Trainium2 NeuronCores (Bass/Tile).

Problem: B=16, LQ=LK=2048, D=128 fp32; per-batch key valid_lens mask.
Sharding: 64 (batch, 512-query-block) units bin-packed into 8 slots x 8
cores by per-batch valid k-tile count, so every core runs an identical
(SPMD) program while skipping fully-masked key tiles entirely.

v2 structure (scores kept transposed; no on-device transposes at all):
  S^T[k, q] = KT_tile.T @ QT_block     (PE, fp16, N=512)
  W^T       = exp(S^T * 1/sqrt(D))     (ScalarE, no bias -- uniform groups)
  rowsum    = DVE pair/chain adds over fully-valid tiles + one PE
              ones-matmul per slot; partially-valid tiles contribute via
              their own PE matmul with the 0/1 mask column as lhsT
  O^T[d, q] += V_tile.T @ W^T          (PE accumulate; V rows >= valid_len
                                        are zeroed host-side so no W mask)
  O^T * (1/rowsum)                     (DVE recip -> GpSimd partition
                                        broadcast -> DVE multiply)
The PE stream is globally software-pipelined: QK of group g+1 is issued
before PV of group g, so ScalarE's exp overlaps PE work instead of
alternating with it. The normalized O^T [d, q] f32 is DMA'd out as-is;
the host transposes each slot during the unshard step (it already packs
transposed Q/K on the way in).
"""

import math

import numpy as np

import concourse.bass as bass
import concourse.mybir as mybir
import concourse.tile as tile
from concourse import bacc
from concourse.bass_utils import run_bass_kernel_spmd

B, LQ, LK, D = 16, 2048, 2048, 128
N_CORES = 8
QB = 512          # query block (one unit) = QB rows of Q
N_SLOTS = (LQ // QB) * B // N_CORES   # 8 slots per core
KT = 128          # key tile
F32 = mybir.dt.float32
FP16 = mybir.dt.float16
SCALE = 1.0 / math.sqrt(D)
G = 2             # k-tiles per exp group (PSUM: st 2x2 + ot 2 + rs 2 = 8)


def _plan(valid_lens):
    """Assign 64 (batch, qblock) units to an 8x8 (slot, core) grid.

    Returns (slot_units, slot_ntiles, masked_from) where
      slot_units[s][c] = (batch, qblock) handled by core c in slot s
      slot_ntiles[s]   = k-tiles processed in slot s (max over cores)
      masked_from[s]   = first k-tile index needing masking on some core
    """
    vl = np.asarray(valid_lens).astype(np.int64)
    ktiles = np.maximum(1, np.ceil(vl / KT).astype(np.int64))
    units = [(int(b), j) for b in range(B) for j in range(LQ // QB)]
    units.sort(key=lambda u: -ktiles[u[0]])
    slot_units, slot_ntiles, masked_from = [], [], []
    for s in range(N_SLOTS):
        chunk = units[s * N_CORES:(s + 1) * N_CORES]
        slot_units.append(chunk)
        slot_ntiles.append(int(max(ktiles[b] for b, _ in chunk)))
        # tile t is fully valid for all cores iff t < min(floor(vl/KT))
        masked_from.append(int(min(vl[b] // KT for b, _ in chunk)))
    return slot_units, slot_ntiles, masked_from


def _pack_inputs(queries, keys, values, valid_lens, slot_units, slot_ntiles):
    """Build per-core packed input arrays (host-side numpy)."""
    vl = np.asarray(valid_lens).astype(np.int64)
    qt = np.ascontiguousarray(np.transpose(queries, (0, 2, 1)))   # [B, D, LQ]
    kt = np.ascontiguousarray(np.transpose(keys, (0, 2, 1)))      # [B, D, LK]
    u_total = sum(slot_ntiles)
    in_maps = []
    for c in range(N_CORES):
        # layouts match SBUF destination exactly -> contiguous DMA
        qt_p = np.zeros((D, N_SLOTS, QB), np.float16)
        kt_p = np.zeros((D, u_total, KT), np.float16)
        v_p = np.zeros((KT, u_total, D), np.float16)
        mask_p = np.zeros((KT, u_total), np.float16)
        off = 0
        for s in range(N_SLOTS):
            b, j = slot_units[s][c]
            qt_p[:, s, :] = qt[b, :, j * QB:(j + 1) * QB]
            n_valid = int(min(slot_ntiles[s], math.ceil(vl[b] / KT)))
            for t in range(n_valid):
                k0 = t * KT
                kt_p[:, off + t, :] = kt[b, :, k0:k0 + KT]
                nv = int(min(KT, vl[b] - k0))
                v_p[:nv, off + t, :] = values[b, k0:k0 + nv, :]
                mask_p[:nv, off + t] = 1.0
            off += slot_ntiles[s]
        in_maps.append({"qt": qt_p, "kt": kt_p, "v": v_p, "mask": mask_p})
    return in_maps, u_total


def build_kernel(slot_ntiles, masked_from, u_total, reps=1):
    nc = bacc.Bacc(None, target_bir_lowering=False, debug=True)
    qt_d = nc.dram_tensor("qt", [D, N_SLOTS, QB], FP16, kind="ExternalInput")
    kt_d = nc.dram_tensor("kt", [D, u_total, KT], FP16, kind="ExternalInput")
    v_d = nc.dram_tensor("v", [KT, u_total, D], FP16, kind="ExternalInput")
    mask_d = nc.dram_tensor("mask", [KT, u_total], FP16, kind="ExternalInput")
    out_d = nc.dram_tensor("out", [N_SLOTS, D, QB], FP16, kind="ExternalOutput")

    n_slots = len(slot_ntiles)

    with tile.TileContext(nc) as tc:
        with (
            tc.tile_pool(name="const", bufs=1) as const,
            tc.tile_pool(name="wt_pool", bufs=4) as wt_pool,
            tc.tile_pool(name="pair_pool", bufs=3) as pair_pool,
            tc.tile_pool(name="acc_pool", bufs=3) as acc_pool,
            tc.tile_pool(name="recip_pool", bufs=2) as recip_pool,
            tc.tile_pool(name="bc_pool", bufs=2) as bc_pool,
            tc.tile_pool(name="onorm_pool", bufs=2) as onorm_pool,
            tc.tile_pool(name="st_psum", bufs=2, space="PSUM") as st_psum,
            tc.tile_pool(name="ot_psum", bufs=2, space="PSUM") as ot_psum,
            tc.tile_pool(name="rs_psum", bufs=2, space="PSUM") as rs_psum,
        ):
            ones = const.tile([128, 1], FP16)
            nc.vector.memset(ones, 1.0)
            # pre-warm the exp table load while input DMA streams in
            warm = const.tile([128, 1], FP16)
            nc.scalar.activation(
                warm, ones, mybir.ActivationFunctionType.Exp, scale=1.0)

            qt_all = const.tile([128, N_SLOTS, QB], FP16)
            mask_all = const.tile([128, u_total], FP16)
            kt_all = const.tile([128, u_total, KT], FP16)
            v_all = const.tile([128, u_total, D], FP16)

            def dma_qt(lo, hi):
                nc.sync.dma_start(
                    out=qt_all[:, lo:hi, :], in_=qt_d[:, lo:hi, :])

            def dma_kv(lo, hi):
                lo, hi = min(lo, u_total), min(hi, u_total)
                if hi <= lo:
                    return
                nc.sync.dma_start(
                    out=kt_all[:, lo:hi, :], in_=kt_d[:, lo:hi, :])
                nc.sync.dma_start(
                    out=v_all[:, lo:hi, :], in_=v_d[:, lo:hi, :])

            # paced input staging: slot-0 Q and the first k-tiles first
            dma_qt(0, 1)
            dma_kv(0, 2)
            dma_kv(2, 4)
            dma_kv(4, 6)
            dma_qt(1, 2)
            dma_kv(6, 10)
            nc.sync.dma_start(out=mask_all, in_=mask_d[:, :])
            dma_kv(10, 14)
            dma_qt(2, 4)
            dma_kv(14, 20)
            dma_kv(20, 28)
            dma_qt(4, N_SLOTS)
            if u_total > 28:
                bnds = [round(28 + i * (u_total - 28) / 3) for i in range(4)]
                for lo, hi in zip(bnds[:-1], bnds[1:]):
                    dma_kv(lo, hi)

            # global group list (pairs of k-tiles, crossing nothing)
            groups = []
            off = 0
            for s, u in enumerate(slot_ntiles):
                t0 = 0
                while t0 < u:
                    gsz = min(G, u - t0)
                    groups.append({"s": s, "t0": t0, "gsz": gsz,
                                   "off": off, "u": u})
                    t0 += gsz
                off += u
            ngrp = len(groups)

            # deferred per-slot work: (due_gi, slot, stage, fn), FIFO in due
            # order. At the final flush all stage-1 chains (ones-mm/recip/
            # broadcast) are emitted before any stage-2 (normalize-mul/DMA)
            # so the remaining tail chains overlap across slots.
            pend = []

            def flush(gi, slot_le=None):
                while pend and (
                    pend[0][0] <= gi
                    or (slot_le is not None and pend[0][1] <= slot_le)
                ):
                    pend.pop(0)[3]()

            def final_flush():
                for entry in [e for e in pend if e[2] == 1]:
                    entry[3]()
                for entry in [e for e in pend if e[2] == 2]:
                    entry[3]()
                pend.clear()

            slot_state = {}

            def _emit_pv_rs(g):
                # PV accumulate for group g, then its masked rowsum matmuls
                # (lhsT = 0/1 mask column). When masked_from == 0 there is
                # no ones-matmul, so the last masked-rs carries stop=True.
                s, u, ss = g["s"], g["u"], slot_state[g["s"]]
                mf = min(masked_from[s], u)
                for tt in range(g["gsz"]):
                    t = g["t0"] + tt
                    nc.tensor.matmul(
                        ss["ot"], v_all[:, g["off"] + t, :],
                        g["wt"][:, tt, :],
                        start=(t == 0), stop=(t == u - 1),
                    )
                for tt in range(g["gsz"]):
                    t = g["t0"] + tt
                    if t >= mf:
                        nc.tensor.matmul(
                            ss["rs"],
                            mask_all[:, g["off"] + t:g["off"] + t + 1],
                            g["wt"][:, tt, :],
                            start=(t == mf),
                            stop=(mf == 0 and t == u - 1),
                        )

            def queue_epilogue(s, ss, gi, gslot):
                def late1(s=s, ss=ss):
                    u = ss["u"]
                    mf = min(masked_from[s], u)
                    if ss["acc"] is not None:
                        nc.tensor.matmul(
                            ss["rs"], ones, ss["acc"],
                            start=(mf >= u), stop=True,
                        )
                    recip = recip_pool.tile([1, QB], F32)
                    nc.vector.reciprocal(recip, ss["rs"])
                    bc = bc_pool.tile([128, QB], F32)
                    nc.gpsimd.partition_broadcast(bc, recip)
                    ss["bc"] = bc

                def late2(s=s, ss=ss):
                    onorm = onorm_pool.tile([128, QB], FP16)
                    nc.vector.tensor_mul(onorm, ss["ot"], ss["bc"])
                    nc.sync.dma_start(out=out_d[s], in_=onorm)

                pend.append((gi + 2, gslot, 1, late1))
                pend.append((gi + 3, gslot, 2, late2))

            gi = 0
            for _rep in range(reps):
                prev = None
                slot_state = {}
                for g in groups:
                    s, u = g["s"], g["u"]
                    gslot = _rep * n_slots + s
                    if g["t0"] == 0:
                        # pool-reuse safety: all deferred work of slot s-2
                        # must be emitted before slot s reuses its buffers
                        flush(-1, slot_le=gslot - 2)
                        slot_state[s] = {
                            "u": u,
                            "ot": ot_psum.tile([128, QB], F32, name="ot"),
                            "rs": rs_psum.tile([1, QB], F32, name="rs"),
                            "acc": None,
                        }
                    ss = slot_state[s]
                    # PE: QK matmuls for this group
                    st = st_psum.tile([128, G, QB], F32)
                    for tt in range(g["gsz"]):
                        t = g["t0"] + tt
                        nc.tensor.matmul(
                            st[:, tt, :], kt_all[:, g["off"] + t, :],
                            qt_all[:, s, :], start=True, stop=True,
                        )
                    # ScalarE: exp on the whole group (no bias)
                    wt = wt_pool.tile([128, G, QB], FP16)
                    nc.scalar.activation(
                        wt[:, :g["gsz"], :], st[:, :g["gsz"], :],
                        mybir.ActivationFunctionType.Exp, scale=SCALE,
                    )
                    g["wt"] = wt
                    # DVE: rowsum tree over fully-valid tiles
                    mf = min(masked_from[s], u)
                    tree_tt = [tt for tt in range(g["gsz"])
                               if g["t0"] + tt < mf]
                    src = None
                    if len(tree_tt) == 2:
                        pr = pair_pool.tile([128, QB], FP16)
                        nc.vector.tensor_add(pr, wt[:, 0, :], wt[:, 1, :])
                        src = pr
                    elif len(tree_tt) == 1:
                        src = wt[:, tree_tt[0], :]
                    if src is not None:
                        if ss["acc"] is None:
                            ss["acc"] = src
                        else:
                            acc2 = acc_pool.tile([128, QB], FP16)
                            nc.vector.tensor_add(acc2, ss["acc"], src)
                            ss["acc"] = acc2
                    # PE: previous group's PV + masked rowsums (pipelined)
                    if prev is not None:
                        _emit_pv_rs(prev)
                    flush(gi)
                    prev = g
                    if g["t0"] + g["gsz"] == u:
                        queue_epilogue(s, ss, gi, gslot)
                    gi += 1
                _emit_pv_rs(prev)
            final_flush()
    nc.finalize()
    return nc


def kernel(queries, keys, values, valid_lens):
    queries = np.ascontiguousarray(np.asarray(queries, dtype=np.float32))
    keys = np.ascontiguousarray(np.asarray(keys, dtype=np.float32))
    values = np.ascontiguousarray(np.asarray(values, dtype=np.float32))
    assert queries.shape == (B, LQ, D), queries.shape
    assert keys.shape == (B, LK, D), keys.shape
    assert values.shape == (B, LK, D), values.shape

    slot_units, slot_ntiles, masked_from = _plan(valid_lens)
    in_maps, u_total = _pack_inputs(
        queries, keys, values, valid_lens, slot_units, slot_ntiles)
    nc = build_kernel(slot_ntiles, masked_from, u_total)
    res = None
    last_exc = None
    for attempt in range(3):
        try:
            res = run_bass_kernel_spmd(nc, in_maps, list(range(N_CORES)))
            break
        except Exception as exc:  # transient NRT/axon failures
            last_exc = exc
            try:
                import jax
                jax.clear_caches()
            except Exception:
                pass
    if res is None:
        raise last_exc

    out = np.empty((B, LQ, D), np.float32)
    for c in range(N_CORES):
        o = res.results[c]["out"]
        for s in range(N_SLOTS):
            b, j = slot_units[s][c]
            out[b, j * QB:(j + 1) * QB, :] = o[s].T
    return out


# revision 23
# speedup vs baseline: 3.8448x; 1.6427x over previous
"""Masked dot-product attention on 8 Trainium2 NeuronCores (Bass/Tile).

Problem: B=16, LQ=LK=2048, D=128 fp32; per-batch key valid_lens mask.
Sharding: 64 (batch, 512-query-block) units bin-packed into 8 slots x 8
cores by per-batch valid k-tile count, so every core runs an identical
(SPMD) program while skipping fully-masked key tiles entirely.

v2 structure (scores kept transposed; no on-device transposes at all):
  S^T[k, q] = KT_tile.T @ QT_block     (PE, fp16, N=512)
  W^T       = exp(S^T * 1/sqrt(D))     (ScalarE, no bias -- uniform groups)
  rowsum    = DVE pair/chain adds over fully-valid tiles + one PE
              ones-matmul per slot; partially-valid tiles contribute via
              their own PE matmul with the 0/1 mask column as lhsT
  O^T[d, q] += V_tile.T @ W^T          (PE accumulate; V rows >= valid_len
                                        are zeroed host-side so no W mask)
  O^T * (1/rowsum)                     (DVE recip -> GpSimd partition
                                        broadcast -> DVE multiply)
The PE stream is globally software-pipelined: QK of group g+1 is issued
before PV of group g, so ScalarE's exp overlaps PE work instead of
alternating with it. The normalized O^T [d, q] f32 is DMA'd out as-is;
the host transposes each slot during the unshard step (it already packs
transposed Q/K on the way in).
"""

import math

import numpy as np

import concourse.bass as bass
import concourse.mybir as mybir
import concourse.tile as tile
from concourse import bacc
from concourse.bass_utils import run_bass_kernel_spmd

B, LQ, LK, D = 16, 2048, 2048, 128
N_CORES = 8
QB = 512          # query block (one unit) = QB rows of Q
N_SLOTS = (LQ // QB) * B // N_CORES   # 8 slots per core
KT = 128          # key tile
F32 = mybir.dt.float32
FP16 = mybir.dt.float16
SCALE = 1.0 / math.sqrt(D)
G = 2             # k-tiles per exp group (PSUM: st 2x2 + ot 2 + rs 2 = 8)


def _plan(valid_lens):
    """Assign 64 (batch, qblock) units to an 8x8 (slot, core) grid.

    Returns (slot_units, slot_ntiles, masked_from) where
      slot_units[s][c] = (batch, qblock) handled by core c in slot s
      slot_ntiles[s]   = k-tiles processed in slot s (max over cores)
      masked_from[s]   = first k-tile index needing masking on some core
    """
    vl = np.asarray(valid_lens).astype(np.int64)
    ktiles = np.maximum(1, np.ceil(vl / KT).astype(np.int64))
    units = [(int(b), j) for b in range(B) for j in range(LQ // QB)]
    units.sort(key=lambda u: -ktiles[u[0]])
    slot_units, slot_ntiles, masked_from = [], [], []
    for s in range(N_SLOTS):
        chunk = units[s * N_CORES:(s + 1) * N_CORES]
        slot_units.append(chunk)
        slot_ntiles.append(int(max(ktiles[b] for b, _ in chunk)))
        # tile t is fully valid for all cores iff t < min(floor(vl/KT))
        masked_from.append(int(min(vl[b] // KT for b, _ in chunk)))
    return slot_units, slot_ntiles, masked_from


def _pack_inputs(queries, keys, values, valid_lens, slot_units, slot_ntiles,
                 fp8=False):
    """Build per-core packed input arrays (host-side numpy)."""
    vl = np.asarray(valid_lens).astype(np.int64)
    qt = np.ascontiguousarray(np.transpose(queries, (0, 2, 1)))   # [B, D, LQ]
    kt = np.ascontiguousarray(np.transpose(keys, (0, 2, 1)))      # [B, D, LK]
    u_total = sum(slot_ntiles)
    in_maps = []
    for c in range(N_CORES):
        # layouts match SBUF destination exactly -> contiguous DMA
        qt_p = np.zeros((D, N_SLOTS, QB), np.float16)
        kt_p = np.zeros((D, u_total, KT), np.float16)
        v_p = np.zeros((KT, u_total, D), np.float16)
        mask_p = np.zeros((KT, u_total), np.float16)
        off = 0
        for s in range(N_SLOTS):
            b, j = slot_units[s][c]
            qt_p[:, s, :] = qt[b, :, j * QB:(j + 1) * QB]
            n_valid = int(min(slot_ntiles[s], math.ceil(vl[b] / KT)))
            for t in range(n_valid):
                k0 = t * KT
                kt_p[:, off + t, :] = kt[b, :, k0:k0 + KT]
                nv = int(min(KT, vl[b] - k0))
                v_p[:nv, off + t, :] = values[b, k0:k0 + nv, :]
                mask_p[:nv, off + t] = 1.0
            off += slot_ntiles[s]
        if not fp8:
            in_maps.append(
                {"qt": qt_p, "kt": kt_p, "v": v_p, "mask": mask_p})
            continue
        import ml_dtypes
        n_pairs = sum((n + 1) // 2 for n in slot_ntiles)
        maskp = np.zeros((KT, n_pairs, 2, 16), ml_dtypes.float8_e4m3fn)
        off = 0
        p = 0
        for s, n in enumerate(slot_ntiles):
            for t0 in range(0, n - 1, 2):
                maskp[:, p, 0, 0] = mask_p[:, off + t0]
                maskp[:, p, 1, 0] = mask_p[:, off + t0 + 1]
                p += 1
            off += n
        in_maps.append({
            "qt": qt_p, "kt": kt_p, "v": v_p, "mask": mask_p,
            "v8": v_p.astype(ml_dtypes.float8_e4m3fn),
            "mask8": mask_p.astype(ml_dtypes.float8_e4m3fn),
            "maskp": maskp,
        })
    return in_maps, u_total


def build_kernel(slot_ntiles, masked_from, u_total, reps=1,
                 do_exp=True, do_pv=True, pv_lag=2, fp8=False,
                 qk_dup=1, exp_dup=1):
    nc = bacc.Bacc(None, target_bir_lowering=False, debug=True)
    qt_d = nc.dram_tensor("qt", [D, N_SLOTS, QB], FP16, kind="ExternalInput")
    kt_d = nc.dram_tensor("kt", [D, u_total, KT], FP16, kind="ExternalInput")
    v_d = nc.dram_tensor("v", [KT, u_total, D], FP16, kind="ExternalInput")
    mask_d = nc.dram_tensor("mask", [KT, u_total], FP16, kind="ExternalInput")
    FP8 = mybir.dt.float8e4
    n_pairs = sum((n + 1) // 2 for n in slot_ntiles)
    if fp8:
        v8_d = nc.dram_tensor(
            "v8", [KT, u_total, D], FP8, kind="ExternalInput")
        mask8_d = nc.dram_tensor(
            "mask8", [KT, u_total], FP8, kind="ExternalInput")
        maskp_d = nc.dram_tensor(
            "maskp", [KT, n_pairs, 2, 16], FP8, kind="ExternalInput")
    out_d = nc.dram_tensor("out", [N_SLOTS, D, QB], FP16, kind="ExternalOutput")

    n_slots = len(slot_ntiles)

    with tile.TileContext(nc) as tc:
        with (
            tc.tile_pool(name="const", bufs=1) as const,
            tc.tile_pool(name="wt_pool", bufs=4) as wt_pool,
            tc.tile_pool(name="pair_pool", bufs=3) as pair_pool,
            tc.tile_pool(name="acc_pool", bufs=3) as acc_pool,
            tc.tile_pool(name="recip_pool", bufs=2) as recip_pool,
            tc.tile_pool(name="bc_pool", bufs=2) as bc_pool,
            tc.tile_pool(name="onorm_pool", bufs=2) as onorm_pool,
            tc.tile_pool(name="st_psum", bufs=2, space="PSUM") as st_psum,
            tc.tile_pool(name="ot_psum", bufs=2, space="PSUM") as ot_psum,
            tc.tile_pool(name="rs_psum", bufs=2, space="PSUM") as rs_psum,
        ):
            ones = const.tile([128, 1], FP16)
            nc.vector.memset(ones, 1.0)
            nbias = const.tile([128, 1], F32)
            nc.vector.memset(nbias, -2.0)
            # pre-warm the exp table load while input DMA streams in
            warm = const.tile([128, 1], FP16)
            nc.scalar.activation(
                warm, ones, mybir.ActivationFunctionType.Exp, scale=1.0)

            qt_all = const.tile([128, N_SLOTS, QB], FP16)
            mask_all = const.tile([128, u_total], FP16 if not fp8 else FP8)
            kt_all = const.tile([128, u_total, KT], FP16)
            v_all = const.tile([128, u_total, D], FP16 if not fp8 else FP8)
            if fp8:
                maskp_all = const.tile([128, n_pairs, 2, 16], FP8)

            def dma_qt(lo, hi):
                nc.sync.dma_start(
                    out=qt_all[:, lo:hi, :], in_=qt_d[:, lo:hi, :])

            def dma_kv(lo, hi):
                lo, hi = min(lo, u_total), min(hi, u_total)
                if hi <= lo:
                    return
                nc.sync.dma_start(
                    out=kt_all[:, lo:hi, :], in_=kt_d[:, lo:hi, :])
                nc.sync.dma_start(
                    out=v_all[:, lo:hi, :],
                    in_=(v8_d if fp8 else v_d)[:, lo:hi, :])

            # paced input staging: slot-0 Q and the first k-tiles first
            dma_qt(0, 1)
            dma_kv(0, 2)
            dma_kv(2, 4)
            dma_kv(4, 6)
            dma_qt(1, 2)
            dma_kv(6, 10)
            nc.sync.dma_start(
                out=mask_all, in_=(mask8_d if fp8 else mask_d)[:, :])
            if fp8:
                nc.sync.dma_start(out=maskp_all, in_=maskp_d[:, :, :, :])
            dma_kv(10, 14)
            dma_qt(2, 4)
            dma_kv(14, 20)
            dma_kv(20, 28)
            dma_qt(4, N_SLOTS)
            if u_total > 28:
                bnds = [round(28 + i * (u_total - 28) / 3) for i in range(4)]
                for lo, hi in zip(bnds[:-1], bnds[1:]):
                    dma_kv(lo, hi)

            # global group list (pairs of k-tiles, crossing nothing)
            groups = []
            off = 0
            pidx = 0
            for s, u in enumerate(slot_ntiles):
                t0 = 0
                while t0 < u:
                    gsz = min(G, u - t0)
                    groups.append({"s": s, "t0": t0, "gsz": gsz,
                                   "off": off, "u": u,
                                   "pidx": pidx if gsz == 2 else None})
                    if gsz == 2:
                        pidx += 1
                    t0 += gsz
                off += u
            ngrp = len(groups)

            # deferred per-slot work: (due_gi, slot, stage, fn), FIFO in due
            # order. At the final flush all stage-1 chains (ones-mm/recip/
            # broadcast) are emitted before any stage-2 (normalize-mul/DMA)
            # so the remaining tail chains overlap across slots.
            pend = []

            def flush(gi, slot_le=None):
                while pend and (
                    pend[0][0] <= gi
                    or (slot_le is not None and pend[0][1] <= slot_le)
                ):
                    pend.pop(0)[3]()

            def final_flush():
                for entry in [e for e in pend if e[2] == 1]:
                    entry[3]()
                for entry in [e for e in pend if e[2] == 2]:
                    entry[3]()
                pend.clear()

            slot_state = {}

            def _emit_pv_rs(g):
                # PV accumulate for group g, then its masked rowsum matmuls
                # (lhsT = 0/1 mask column). When masked_from == 0 there is
                # no ones-matmul, so the last masked-rs carries stop=True.
                s, u, ss = g["s"], g["u"], slot_state[g["s"]]
                t0, gsz, off_g = g["t0"], g["gsz"], g["off"]
                if fp8:
                    last = (t0 + gsz == u)
                    wt_pair = g["wt"][:, :, :]
                    if do_pv:
                        if gsz == 2:
                            nc.tensor.matmul(
                                ss["ot"],
                                v_all[:, off_g + t0:off_g + t0 + 2, :],
                                wt_pair,
                                start=(t0 == 0), stop=last,
                                perf_mode=mybir.MatmulPerfMode.DoubleRow,
                            )
                        else:
                            nc.tensor.matmul(
                                ss["ot"], v_all[:, off_g + t0, :],
                                g["wt"][:, 0, :],
                                start=(t0 == 0), stop=last,
                            )
                    if gsz == 2:
                        nc.tensor.matmul(
                            ss["rs"],
                            maskp_all[:, g["pidx"], :, 0:1],
                            wt_pair,
                            start=(t0 == 0), stop=last,
                            perf_mode=mybir.MatmulPerfMode.DoubleRow,
                        )
                    else:
                        nc.tensor.matmul(
                            ss["rs"],
                            mask_all[:, off_g + t0:off_g + t0 + 1],
                            g["wt"][:, 0, :],
                            start=(t0 == 0), stop=last,
                        )
                    return
                mf = min(masked_from[s], u)
                if do_pv:
                    for tt in range(gsz):
                        t = t0 + tt
                        nc.tensor.matmul(
                            ss["ot"], v_all[:, off_g + t, :],
                            g["wt"][:, tt, :],
                            start=(t == 0), stop=(t == u - 1),
                        )
                for tt in range(gsz):
                    t = t0 + tt
                    if t >= mf:
                        nc.tensor.matmul(
                            ss["rs"],
                            mask_all[:, off_g + t:off_g + t + 1],
                            g["wt"][:, tt, :],
                            start=(t == mf),
                            stop=(mf == 0 and t == u - 1),
                        )

            def queue_epilogue(s, ss, gi, gslot):
                def late1(s=s, ss=ss):
                    u = ss["u"]
                    mf = min(masked_from[s], u)
                    if not fp8 and ss["acc"] is not None:
                        nc.tensor.matmul(
                            ss["rs"], ones, ss["acc"],
                            start=(mf >= u), stop=True,
                        )
                    recip = recip_pool.tile([1, QB], F32)
                    nc.vector.reciprocal(recip, ss["rs"])
                    bc = bc_pool.tile([128, QB], F32)
                    nc.gpsimd.partition_broadcast(bc, recip)
                    ss["bc"] = bc

                def late2(s=s, ss=ss):
                    onorm = onorm_pool.tile([128, QB], FP16)
                    nc.vector.tensor_mul(onorm, ss["ot"], ss["bc"])
                    nc.sync.dma_start(out=out_d[s], in_=onorm)

                pend.append((gi + 1 + pv_lag, gslot, 1, late1))
                pend.append((gi + 2 + pv_lag, gslot, 2, late2))

            gi = 0
            for _rep in range(reps):
                lagq = []
                slot_state = {}
                for g in groups:
                    s, u = g["s"], g["u"]
                    gslot = _rep * n_slots + s
                    if g["t0"] == 0:
                        # pool-reuse safety: all deferred work of slot s-2
                        # must be emitted before slot s reuses its buffers
                        flush(-1, slot_le=gslot - 2)
                        slot_state[s] = {
                            "u": u,
                            "ot": ot_psum.tile([128, QB], F32, name="ot"),
                            "rs": rs_psum.tile([1, QB], F32, name="rs"),
                            "acc": None,
                        }
                    ss = slot_state[s]
                    # PE: QK matmuls for this group
                    st = st_psum.tile([128, G, QB], F32)
                    for _dup in range(qk_dup):
                        for tt in range(g["gsz"]):
                            t = g["t0"] + tt
                            nc.tensor.matmul(
                                st[:, tt, :], kt_all[:, g["off"] + t, :],
                                qt_all[:, s, :], start=True, stop=True,
                            )
                    # ScalarE: exp on the whole group (no bias)
                    wt = wt_pool.tile([128, G, QB], FP16 if not fp8 else FP8)
                    if do_exp:
                        for _dup in range(exp_dup):
                            # fp8: bias -2 keeps exp under the e4m3 max of
                            # 448 (max |score| ~ 6.3 over this tensor size);
                            # the uniform e^-2 factor cancels in normalize
                            if fp8:
                                nc.scalar.activation(
                                    wt[:, :g["gsz"], :], st[:, :g["gsz"], :],
                                    mybir.ActivationFunctionType.Exp,
                                    scale=SCALE, bias=nbias,
                                )
                            else:
                                nc.scalar.activation(
                                    wt[:, :g["gsz"], :], st[:, :g["gsz"], :],
                                    mybir.ActivationFunctionType.Exp,
                                    scale=SCALE,
                                )
                    g["wt"] = wt
                    # DVE: rowsum tree over fully-valid tiles
                    mf = 0 if fp8 else min(masked_from[s], u)
                    tree_tt = [tt for tt in range(g["gsz"])
                               if g["t0"] + tt < mf]
                    src = None
                    if len(tree_tt) == 2:
                        pr = pair_pool.tile([128, QB], FP16)
                        nc.vector.tensor_add(pr, wt[:, 0, :], wt[:, 1, :])
                        src = pr
                    elif len(tree_tt) == 1:
                        src = wt[:, tree_tt[0], :]
                    if src is not None:
                        if ss["acc"] is None:
                            ss["acc"] = src
                        else:
                            acc2 = acc_pool.tile([128, QB], FP16)
                            nc.vector.tensor_add(acc2, ss["acc"], src)
                            ss["acc"] = acc2
                    # PE: lagged groups' PV + masked rowsums (pipelined)
                    lagq.append(g)
                    if len(lagq) > pv_lag:
                        _emit_pv_rs(lagq.pop(0))
                    flush(gi)
                    if g["t0"] + g["gsz"] == u:
                        queue_epilogue(s, ss, gi, gslot)
                    gi += 1
                for gq in lagq:
                    _emit_pv_rs(gq)
                lagq = []
            final_flush()
    nc.finalize()
    return nc


def kernel(queries, keys, values, valid_lens):
    queries = np.ascontiguousarray(np.asarray(queries, dtype=np.float32))
    keys = np.ascontiguousarray(np.asarray(keys, dtype=np.float32))
    values = np.ascontiguousarray(np.asarray(values, dtype=np.float32))
    assert queries.shape == (B, LQ, D), queries.shape
    assert keys.shape == (B, LK, D), keys.shape
    assert values.shape == (B, LK, D), values.shape

    slot_units, slot_ntiles, masked_from = _plan(valid_lens)
    in_maps, u_total = _pack_inputs(
        queries, keys, values, valid_lens, slot_units, slot_ntiles)
    nc = build_kernel(slot_ntiles, masked_from, u_total)
    res = None
    last_exc = None
    for attempt in range(3):
        try:
            res = run_bass_kernel_spmd(nc, in_maps, list(range(N_CORES)))
            break
        except Exception as exc:  # transient NRT/axon failures
            last_exc = exc
            try:
                import jax
                jax.clear_caches()
            except Exception:
                pass
    if res is None:
        raise last_exc

    out = np.empty((B, LQ, D), np.float32)
    for c in range(N_CORES):
        o = res.results[c]["out"]
        for s in range(N_SLOTS):
            b, j = slot_units[s][c]
            out[b, j * QB:(j + 1) * QB, :] = o[s].T
    return out
